# revision 12
# baseline (speedup 1.0000x reference)
"""Trainium2 Bass kernel for nn_CompatibleLearningLoss (MoCo-style queue contrastive loss).

Strategy: shard the queue dimension (Q=32768) across 8 NeuronCores (4096 rows each).
Each core computes its slice of the three matmuls
    old_embeds  @ feat_queue_shard.T   -> weight
    new_e       @ feat_queue_shard.T   -> scores1
    new_logits  @ logit_queue_shard.T  -> scores2
and reduces per-row partial softmax statistics (chunk max, exp-sum vs chunk max,
masked-weighted raw sums) into a [128, 128] stats tile.  The host combines the
partials in float64 and produces the two scalar losses.

The circular queue scatter, the new_embeds normalization, the label mask and the
small-matrix transposes are done host-side (they are O(MB) vs the 1GB queue reads).
"""

from contextlib import nullcontext

import numpy as np

import concourse.bass as bass
import concourse.tile as tile
from concourse import mybir
from concourse.bass_utils import run_bass_kernel_spmd
from concourse.masks import make_identity
from concourse.vector_clock import ScopedClock

N = 128      # batch
D = 512      # embed dim
C = 8192     # logit dim
Q = 32768    # queue length
N_CORES = 8
QS = Q // N_CORES          # 4096 queue rows per core
EPS = 1e-12

F32 = mybir.dt.float32

# stats tile column layout (per core, [128, 128] f32)
# cols 0:8    m1 parts   (feat-path chunk maxes, 8 chunks of 512)
# cols 8:16   z1 parts   (feat-path sum exp(s - chunk max))
# cols 16:24  a1 parts   (feat-path sum maskw * s_raw)
# cols 24:32  W  parts   (sum maskw)
# cols 32:64  m2 parts   (logit-path chunk maxes, 32 chunks of 128)
# cols 64:96  z2 parts
# cols 96:128 a2 parts
A_CHUNKS = 8    # feat path: 8 chunks of 512 q
B_CHUNKS = 32   # logit path: 32 chunks of 128 q


def _split_excess_waits(nc: bass.Bass, limit: int = 1) -> None:
    """This walrus build rejects instructions carrying more than one sync wait
    ("Too many sync wait commands").  Tile's sem-assignment freely attaches
    several.  Move excess waits onto same-engine nops inserted right before
    the offending instruction (queue order makes that equivalent)."""
    for f in nc.m.functions:
        for bb in f.blocks:
            insts = bb.instructions
            insertions = []
            for idx, inst in enumerate(insts):
                si = inst.sync_info
                if si is None:
                    continue
                cap = 2 if isinstance(inst, mybir.InstEventSemaphore) else limit
                waits = list(si.on_wait)
                if len(waits) <= cap:
                    continue
                keep = waits[:cap]
                excess = waits[cap:]
                si.on_wait = keep
                nops = []
                for w in excess:
                    nop = mybir.InstNoOp(
                        name=nc.get_next_instruction_name(), ins=[], outs=[]
                    )
                    nop.engine = inst.engine
                    nop.sync_info = mybir.SyncInfo(on_wait=[w], on_update=[])
                    nc.register_instruction(nop, overwrite=True)
                    nops.append(nop)
                insertions.append((idx, nops))
            for idx, nops in reversed(insertions):
                for nop in reversed(nops):
                    bb.instructions.insert(idx, nop)


class PatchedTileContext(tile.TileContext):
    """Work around the 1-sync-wait-per-instruction cap in this walrus build:
    the stock TileContext tail drain carries one wait per outstanding proc,
    which codegen rejects ("Too many sync wait commands").  Split the waits
    across single-wait SP nops instead."""

    def _drain_and_barrier(self, tick_clock, wait_clock):
        drain_inst = self.nc.sync.drain()
        wait_clock.add_sem_waits(
            drain_inst.ins, ScopedClock({None: tick_clock.global_clock})
        )
        si = drain_inst.ins.sync_info
        if si is not None and len(si.on_wait) > 1:
            waits = list(si.on_wait)
            si.on_wait = [waits[0]]
            for w in waits[1:]:
                nop = self.nc.sync.nop(nofuse=True, hint="drain_wait_split")
                nop.ins.sync_info = mybir.SyncInfo(on_wait=[w], on_update=[])
        self.nc.all_engine_barrier()
        assert self.sems is not None
        popped = self.nc._tile_sem_poison_stack.pop()
        assert popped is self._sem_poison
        self.nc.clear_and_free_semaphores(list(self.sems.allocated().values()))
        self.nc.all_engine_barrier()


def _build_program(repeat: int = 1) -> bass.Bass:
    nc = bass.Bass()

    feat = nc.dram_tensor("feat", [QS, D], F32, kind="ExternalInput")
    logit = nc.dram_tensor("logit", [QS, C], F32, kind="ExternalInput")
    maskh = nc.dram_tensor("maskh", [N, QS], F32, kind="ExternalInput")
    neT = nc.dram_tensor("neT", [D, N], F32, kind="ExternalInput")
    oeT = nc.dram_tensor("oeT", [D, N], F32, kind="ExternalInput")
    nlT = nc.dram_tensor("nlT", [C, N], F32, kind="ExternalInput")
    stats = nc.dram_tensor("stats", [N, 128], F32, kind="ExternalOutput")

    AX = mybir.AxisListType
    OP = mybir.AluOpType
    ACT = mybir.ActivationFunctionType

    with PatchedTileContext(nc) as tc:
        with (
            tc.tile_pool(name="const", bufs=1) as const,
            tc.tile_pool(name="small", bufs=4) as small,
            tc.tile_pool(name="scr", bufs=3) as scrp,
            tc.tile_pool(name="tsl", bufs=3) as tslp,
            tc.tile_pool(name="psum_t", bufs=3, space="PSUM") as psum_t,
        ):
            ident = const.tile([128, 128], F32)
            make_identity(nc, ident)

            # replicated small operands, pre-transposed host-side
            neT_sb = const.tile([128, D // 128, N], F32)
            nc.sync.dma_start(out=neT_sb, in_=neT.rearrange("(c p) n -> p c n", p=128))
            oeT_sb = const.tile([128, D // 128, N], F32)
            nc.sync.dma_start(out=oeT_sb, in_=oeT.rearrange("(c p) n -> p c n", p=128))
            nlT_sb = const.tile([128, C // 128, N], F32)
            nc.sync.dma_start(out=nlT_sb, in_=nlT.rearrange("(c p) n -> p c n", p=128))

            mw_sb = const.tile([N, QS], F32)
            out_sb = const.tile([N, 128], F32)

            loop_cm = tc.For_i(0, repeat, 1) if repeat > 1 else nullcontext()
            with loop_cm:
                # 0.5*mask, becomes maskw = 0.5*(old_outputs+1)*mask in place
                nc.sync.dma_start(out=mw_sb, in_=maskh[:, :])

                # ------------ Phase A: feat path (512-wide q chunks) ----------
                with (
                    tc.tile_pool(name="natA", bufs=2) as natA_pool,
                    tc.tile_pool(name="psum_a", bufs=2, space="PSUM") as psum_a,
                ):
                    for qc in range(A_CHUNKS):
                        nat = natA_pool.tile([128, 4, D], F32)
                        nc.sync.dma_start(
                            out=nat,
                            in_=feat[qc * 512 : (qc + 1) * 512, :].rearrange(
                                "(s p) d -> p s d", p=128
                            ),
                        )
                        ps1 = psum_a.tile([128, 512], F32, tag="ps1")
                        psw = psum_a.tile([128, 512], F32, tag="psw")
                        for dc in range(4):
                            pt = psum_t.tile([128, 512], F32, tag="pt")
                            for qs in range(4):
                                nc.tensor.matmul(
                                    pt[:, qs * 128 : (qs + 1) * 128],
                                    nat[:, qs, dc * 128 : (dc + 1) * 128],
                                    ident,
                                    is_transpose=True,
                                    start=(qs == 0),
                                    stop=(qs == 3),
                                )
                            tsl = tslp.tile([128, 512], F32, tag="tsl")
                            if dc % 2 == 0:
                                nc.vector.tensor_copy(out=tsl, in_=pt)
                            else:
                                nc.scalar.copy(out=tsl, in_=pt)
                            nc.tensor.matmul(
                                ps1, neT_sb[:, dc, :], tsl,
                                start=(dc == 0), stop=(dc == 3),
                            )
                            nc.tensor.matmul(
                                psw, oeT_sb[:, dc, :], tsl,
                                start=(dc == 0), stop=(dc == 3),
                            )
                        qslice = slice(qc * 512, (qc + 1) * 512)
                        # maskw = (w + 1) * 0.5mask, in place over the 0.5mask
                        nc.vector.scalar_tensor_tensor(
                            out=mw_sb[:, qslice], in0=psw, scalar=1.0,
                            in1=mw_sb[:, qslice], op0=OP.add, op1=OP.mult,
                        )
                        # W partial
                        nc.vector.tensor_reduce(
                            out=out_sb[:, 24 + qc : 25 + qc], in_=mw_sb[:, qslice],
                            axis=AX.X, op=OP.add,
                        )
                        # a1 partial = sum maskw * s1_raw
                        scr = scrp.tile([128, 512], F32, tag="scr")
                        nc.vector.scalar_tensor_tensor(
                            out=scr, in0=ps1, scalar=1.0, in1=mw_sb[:, qslice],
                            op0=OP.mult, op1=OP.mult,
                            accum_out=out_sb[:, 16 + qc : 17 + qc],
                        )
                        # m1 partial (chunk max)
                        nc.vector.tensor_reduce(
                            out=out_sb[:, qc : qc + 1], in_=ps1, axis=AX.X, op=OP.max,
                        )
                        negm = small.tile([128, 1], F32, tag="negm")
                        nc.vector.tensor_scalar_mul(
                            out=negm, in0=out_sb[:, qc : qc + 1], scalar1=-1.0
                        )
                        # z1 partial = sum exp(s1 - chunk max)
                        escr = scrp.tile([128, 512], F32, tag="escr")
                        nc.scalar.activation(
                            out=escr, in_=ps1, func=ACT.Exp, bias=negm, scale=1.0,
                            accum_out=out_sb[:, 8 + qc : 9 + qc],
                        )

                # ------------ Phase B: logit path (128-wide q blocks) ---------
                with (
                    tc.tile_pool(name="natB", bufs=3) as natB_pool,
                    tc.tile_pool(name="psum_b", bufs=2, space="PSUM") as psum_b,
                ):
                    for qb in range(B_CHUNKS):
                        nat = natB_pool.tile([128, C], F32)
                        nc.sync.dma_start(
                            out=nat, in_=logit[qb * 128 : (qb + 1) * 128, :]
                        )
                        ps2 = psum_b.tile([128, 128], F32, tag="ps2")
                        for g in range(16):  # groups of 4 c-chunks
                            pt = psum_t.tile([128, 512], F32, tag="pt")
                            for k in range(4):
                                cb = g * 4 + k
                                nc.tensor.matmul(
                                    pt[:, k * 128 : (k + 1) * 128],
                                    nat[:, cb * 128 : (cb + 1) * 128],
                                    ident,
                                    is_transpose=True,
                                    start=(k == 0),
                                    stop=(k == 3),
                                )
                            tsl = tslp.tile([128, 512], F32, tag="tsl")
                            if g % 2 == 0:
                                nc.vector.tensor_copy(out=tsl, in_=pt)
                            else:
                                nc.scalar.copy(out=tsl, in_=pt)
                            for k in range(4):
                                cb = g * 4 + k
                                nc.tensor.matmul(
                                    ps2, nlT_sb[:, cb, :],
                                    tsl[:, k * 128 : (k + 1) * 128],
                                    start=(cb == 0), stop=(cb == C // 128 - 1),
                                )
                        qslice = slice(qb * 128, (qb + 1) * 128)
                        # a2 partial
                        scr = scrp.tile([128, 512], F32, tag="scr")
                        nc.vector.scalar_tensor_tensor(
                            out=scr[:, :128], in0=ps2, scalar=1.0,
                            in1=mw_sb[:, qslice], op0=OP.mult, op1=OP.mult,
                            accum_out=out_sb[:, 96 + qb : 97 + qb],
                        )
                        # m2 partial
                        nc.vector.tensor_reduce(
                            out=out_sb[:, 32 + qb : 33 + qb], in_=ps2,
                            axis=AX.X, op=OP.max,
                        )
                        negm = small.tile([128, 1], F32, tag="negm")
                        nc.vector.tensor_scalar_mul(
                            out=negm, in0=out_sb[:, 32 + qb : 33 + qb], scalar1=-1.0
                        )
                        # z2 partial
                        escr = scrp.tile([128, 512], F32, tag="escr")
                        nc.scalar.activation(
                            out=escr[:, :128], in_=ps2, func=ACT.Exp, bias=negm,
                            scale=1.0, accum_out=out_sb[:, 64 + qb : 65 + qb],
                        )

            nc.sync.dma_start(out=stats[:, :], in_=out_sb)

    _split_excess_waits(nc)
    return nc


_PROGRAM: bass.Bass | None = None
LAST_RESULTS = None  # BassKernelResults of the most recent run (for profiling)


def _get_program() -> bass.Bass:
    global _PROGRAM
    if _PROGRAM is None:
        _PROGRAM = _build_program()
    return _PROGRAM


def host_prep(old_embeds, old_logits, new_embeds, new_logits, labels,
              feat_queue, logit_queue, queue_labels, header):
    """Scatter + normalize + mask on host; returns per-core in_maps and M."""
    old_embeds = np.asarray(old_embeds, dtype=np.float32)
    old_logits = np.asarray(old_logits, dtype=np.float32)
    new_embeds = np.asarray(new_embeds, dtype=np.float32)
    new_logits = np.asarray(new_logits, dtype=np.float32)
    feat_queue = np.array(feat_queue, dtype=np.float32)   # copies (scattered below)
    logit_queue = np.array(logit_queue, dtype=np.float32)
    labels_np = np.asarray(labels).astype(np.int64)
    queue_labels_np = np.asarray(queue_labels).astype(np.int64)
    hdr = int(np.asarray(header))

    n = old_embeds.shape[0]
    q = feat_queue.shape[0]
    assert (n, q) == (N, Q)

    # circular queue scatter
    idx = (hdr + np.arange(n)) % q
    feat_queue[idx] = old_embeds
    logit_queue[idx] = old_logits
    queue_labels_np[idx] = labels_np

    # normalize new_embeds (f64 intermediate, f32 result)
    ne64 = new_embeds.astype(np.float64)
    norm = np.sqrt((ne64 * ne64).sum(axis=1, keepdims=True))
    new_e = (ne64 / np.maximum(norm, EPS)).astype(np.float32)

    # label mask (host): maskh = 0.5 * mask
    mask = (queue_labels_np[None, :] == labels_np[:, None])
    M = mask.sum(axis=1).astype(np.float64)               # [N], >= 1 by construction
    maskh = 0.5 * mask.astype(np.float32)

    neT = np.ascontiguousarray(new_e.T)                   # [D, N]
    oeT = np.ascontiguousarray(old_embeds.T)              # [D, N]
    nlT = np.ascontiguousarray(new_logits.T)              # [C, N]

    in_maps = []
    for d in range(N_CORES):
        sl = slice(d * QS, (d + 1) * QS)
        in_maps.append({
            "feat": np.ascontiguousarray(feat_queue[sl]),
            "logit": np.ascontiguousarray(logit_queue[sl]),
            "maskh": np.ascontiguousarray(maskh[:, sl]),
            "neT": neT,
            "oeT": oeT,
            "nlT": nlT,
        })
    return in_maps, M


def combine_stats(parts: np.ndarray, M: np.ndarray):
    """parts: [n_cores, 128, 128] f32 stats tiles -> (l1, l2) f32 scalars."""
    parts = parts.astype(np.float64)
    m1p = parts[:, :, 0:8]
    z1p = parts[:, :, 8:16]
    a1p = parts[:, :, 16:24]
    wp = parts[:, :, 24:32]
    m2p = parts[:, :, 32:64]
    z2p = parts[:, :, 64:96]
    a2p = parts[:, :, 96:128]

    W = wp.sum(axis=(0, 2))                               # [N]
    A1 = a1p.sum(axis=(0, 2))
    A2 = a2p.sum(axis=(0, 2))
    m1 = m1p.max(axis=(0, 2))
    m2 = m2p.max(axis=(0, 2))
    Z1 = (z1p * np.exp(m1p - m1[None, :, None])).sum(axis=(0, 2))
    Z2 = (z2p * np.exp(m2p - m2[None, :, None])).sum(axis=(0, 2))

    # sum_j maskw * log_prob = A_raw - (m + log Z) * W ; divide by count, mean, negate
    l1 = -np.mean((A1 - (m1 + np.log(Z1)) * W) / M)
    l2 = -np.mean((A2 - (m2 + np.log(Z2)) * W) / M)
    return (np.float32(l1), np.float32(l2))


def kernel(old_embeds, old_logits, new_embeds, new_logits, labels,
           feat_queue, logit_queue, queue_labels, header):
    global LAST_RESULTS
    in_maps, M = host_prep(
        old_embeds, old_logits, new_embeds, new_logits, labels,
        feat_queue, logit_queue, queue_labels, header,
    )
    nc = _get_program()
    LAST_RESULTS = run_bass_kernel_spmd(nc, in_maps, list(range(N_CORES)))
    parts = np.stack([LAST_RESULTS.results[d]["stats"] for d in range(N_CORES)])
    return combine_stats(parts, M)


# revision 29
# speedup vs baseline: 1.6923x; 1.6923x over previous
"""Trainium2 Bass kernel for nn_CompatibleLearningLoss (MoCo-style queue contrastive loss).

Strategy: shard the queue dimension (Q=32768) across 8 NeuronCores (4096 rows each).
Each core computes its slice of the three matmuls
    old_embeds  @ feat_queue_shard.T   -> weight
    new_e       @ feat_queue_shard.T   -> scores1
    new_logits  @ logit_queue_shard.T  -> scores2
and reduces per-row partial softmax statistics (chunk max, exp-sum vs chunk max,
masked-weighted raw sums) into a [128, 128] stats tile.  The host combines the
partials in float64 and produces the two scalar losses.

The circular queue scatter, the new_embeds normalization, the label mask and the
small-matrix transposes are done host-side (they are O(MB) vs the 1GB queue reads).
"""

from contextlib import nullcontext

import numpy as np

import concourse.bass as bass
import concourse.tile as tile
from concourse import mybir
from concourse.bass_utils import run_bass_kernel_spmd
from concourse.masks import make_identity
from concourse.vector_clock import ScopedClock

N = 128      # batch
D = 512      # embed dim
C = 8192     # logit dim
Q = 32768    # queue length
N_CORES = 8
QS = Q // N_CORES          # 4096 queue rows per core
EPS = 1e-12

F32 = mybir.dt.float32

# stats tile column layout (per core, [128, 128] f32)
# cols 0:8      m1 parts   (feat-path chunk maxes, 8 chunks of 512)
# cols 8:16     z1 parts   (feat-path sum exp(s - chunk max))
# cols 16:24    a1 parts   (feat-path sum maskw * s_raw)
# cols 24:32    W  parts   (sum maskw)
# cols 32:32+B  m2 parts   (logit-path chunk maxes, B_CHUNKS chunks)
# cols 64:64+B  z2 parts
# cols 96:96+B  a2 parts
A_CHUNKS = 8    # feat path: 8 chunks of 512 q
B_CHUNKS = 16   # logit path: 16 chunks of 256 q

# precision/perf knobs (validated against the fp32 reference on hardware)
BF16 = mybir.dt.bfloat16
F32R = mybir.dt.float32r
# fp32r (reduced-precision fp32 multiply) for the whole logit path: transposes
# at 1.5 cyc/row and N=256 matmuls at 1 cyc/row vs fp32's 2.0/4.0.  The BIR
# verifier requires dtype consistency along every producer->consumer edge, so
# the full chain (DRAM logit, nat, identity, psum transpose out, tbuf, nlT)
# is declared float32r.  Storage is identical to fp32.
B_DT = F32R


def _split_excess_waits(nc: bass.Bass, limit: int = 1) -> None:
    """This walrus build rejects instructions carrying more than one sync wait
    ("Too many sync wait commands").  Tile's sem-assignment freely attaches
    several.  Move excess waits onto same-engine nops inserted right before
    the offending instruction (queue order makes that equivalent)."""
    for f in nc.m.functions:
        for bb in f.blocks:
            insts = bb.instructions
            insertions = []
            for idx, inst in enumerate(insts):
                si = inst.sync_info
                if si is None:
                    continue
                cap = 2 if isinstance(inst, mybir.InstEventSemaphore) else limit
                waits = list(si.on_wait)
                if len(waits) <= cap:
                    continue
                keep = waits[:cap]
                excess = waits[cap:]
                si.on_wait = keep
                nops = []
                for w in excess:
                    nop = mybir.InstNoOp(
                        name=nc.get_next_instruction_name(), ins=[], outs=[]
                    )
                    nop.engine = inst.engine
                    nop.sync_info = mybir.SyncInfo(on_wait=[w], on_update=[])
                    nc.register_instruction(nop, overwrite=True)
                    nops.append(nop)
                insertions.append((idx, nops))
            for idx, nops in reversed(insertions):
                for nop in reversed(nops):
                    bb.instructions.insert(idx, nop)


class PatchedTileContext(tile.TileContext):
    """Work around the 1-sync-wait-per-instruction cap in this walrus build:
    the stock TileContext tail drain carries one wait per outstanding proc,
    which codegen rejects ("Too many sync wait commands").  Split the waits
    across single-wait SP nops instead."""

    def _drain_and_barrier(self, tick_clock, wait_clock):
        drain_inst = self.nc.sync.drain()
        wait_clock.add_sem_waits(
            drain_inst.ins, ScopedClock({None: tick_clock.global_clock})
        )
        si = drain_inst.ins.sync_info
        if si is not None and len(si.on_wait) > 1:
            waits = list(si.on_wait)
            si.on_wait = [waits[0]]
            for w in waits[1:]:
                nop = self.nc.sync.nop(nofuse=True, hint="drain_wait_split")
                nop.ins.sync_info = mybir.SyncInfo(on_wait=[w], on_update=[])
        self.nc.all_engine_barrier()
        assert self.sems is not None
        popped = self.nc._tile_sem_poison_stack.pop()
        assert popped is self._sem_poison
        self.nc.clear_and_free_semaphores(list(self.sems.allocated().values()))
        self.nc.all_engine_barrier()


def _build_program(repeat: int = 1) -> bass.Bass:
    nc = bass.Bass()

    feat = nc.dram_tensor("feat", [QS, D], F32, kind="ExternalInput")
    logit = nc.dram_tensor("logit", [QS, C], B_DT, kind="ExternalInput")
    maskh = nc.dram_tensor("maskh", [N, QS], F32, kind="ExternalInput")
    neT = nc.dram_tensor("neT", [D, N], F32, kind="ExternalInput")
    oeT = nc.dram_tensor("oeT", [D, N], F32, kind="ExternalInput")
    nlT = nc.dram_tensor("nlT", [C, N], B_DT, kind="ExternalInput")
    identB = nc.dram_tensor("identB", [128, 128], B_DT, kind="ExternalInput")
    stats = nc.dram_tensor("stats", [N, 128], F32, kind="ExternalOutput")

    AX = mybir.AxisListType
    OP = mybir.AluOpType
    ACT = mybir.ActivationFunctionType

    with PatchedTileContext(nc) as tc:
        with (
            tc.tile_pool(name="const", bufs=1) as const,
            tc.tile_pool(name="small", bufs=4) as small,
            tc.tile_pool(name="scr", bufs=2) as scrp,
            tc.tile_pool(name="tsl", bufs=3) as tslp,
            tc.tile_pool(name="psum_t", bufs=3, space="PSUM") as psum_t,
        ):
            ident = const.tile([128, 128], F32)
            make_identity(nc, ident)

            # replicated small operands, pre-transposed host-side
            neT_sb = const.tile([128, D // 128, N], F32)
            nc.sync.dma_start(out=neT_sb, in_=neT.rearrange("(c p) n -> p c n", p=128))
            oeT_sb = const.tile([128, D // 128, N], F32)
            nc.sync.dma_start(out=oeT_sb, in_=oeT.rearrange("(c p) n -> p c n", p=128))
            nlT_sb = const.tile([128, C // 128, N], B_DT)
            nc.sync.dma_start(out=nlT_sb, in_=nlT.rearrange("(c p) n -> p c n", p=128))
            identB_sb = const.tile([128, 128], B_DT)
            nc.sync.dma_start(out=identB_sb, in_=identB[:, :])

            mw_sb = const.tile([N, QS], F32)
            out_sb = const.tile([N, 128], F32)

            loop_cm = tc.For_i(0, repeat, 1) if repeat > 1 else nullcontext()
            with loop_cm:
                # 0.5*mask, becomes maskw = 0.5*(old_outputs+1)*mask in place
                nc.sync.dma_start(out=mw_sb, in_=maskh[:, :])

                # ------------ Phase A: feat path (512-wide q chunks) ----------
                with (
                    tc.tile_pool(name="natA", bufs=2) as natA_pool,
                    tc.tile_pool(name="psum_a", bufs=2, space="PSUM") as psum_a,
                ):
                    for qc in range(A_CHUNKS):
                        nat = natA_pool.tile([128, 4, D], F32)
                        nc.sync.dma_start(
                            out=nat,
                            in_=feat[qc * 512 : (qc + 1) * 512, :].rearrange(
                                "(s p) d -> p s d", p=128
                            ),
                        )
                        ps1 = psum_a.tile([128, 512], F32, tag="ps1")
                        psw = psum_a.tile([128, 512], F32, tag="psw")
                        for dc in range(4):
                            pt = psum_t.tile([128, 512], F32, tag="pt")
                            for qs in range(4):
                                nc.tensor.matmul(
                                    pt[:, qs * 128 : (qs + 1) * 128],
                                    nat[:, qs, dc * 128 : (dc + 1) * 128],
                                    ident,
                                    is_transpose=True,
                                    start=(qs == 0),
                                    stop=(qs == 3),
                                )
                            tsl = tslp.tile([128, 512], F32, tag="tsl")
                            if dc % 2 == 0:
                                nc.vector.tensor_copy(out=tsl, in_=pt)
                            else:
                                nc.scalar.copy(out=tsl, in_=pt)
                            nc.tensor.matmul(
                                ps1, neT_sb[:, dc, :], tsl,
                                start=(dc == 0), stop=(dc == 3),
                            )
                            nc.tensor.matmul(
                                psw, oeT_sb[:, dc, :], tsl,
                                start=(dc == 0), stop=(dc == 3),
                            )
                        qslice = slice(qc * 512, (qc + 1) * 512)
                        # maskw = (w + 1) * 0.5mask, in place over the 0.5mask
                        nc.vector.scalar_tensor_tensor(
                            out=mw_sb[:, qslice], in0=psw, scalar=1.0,
                            in1=mw_sb[:, qslice], op0=OP.add, op1=OP.mult,
                        )
                        # W partial
                        nc.vector.tensor_reduce(
                            out=out_sb[:, 24 + qc : 25 + qc], in_=mw_sb[:, qslice],
                            axis=AX.X, op=OP.add,
                        )
                        # a1 partial = sum maskw * s1_raw
                        scr = scrp.tile([128, 512], F32, tag="scr")
                        nc.vector.scalar_tensor_tensor(
                            out=scr, in0=ps1, scalar=1.0, in1=mw_sb[:, qslice],
                            op0=OP.mult, op1=OP.mult,
                            accum_out=out_sb[:, 16 + qc : 17 + qc],
                        )
                        # m1 partial (chunk max)
                        nc.vector.tensor_reduce(
                            out=out_sb[:, qc : qc + 1], in_=ps1, axis=AX.X, op=OP.max,
                        )
                        negm = small.tile([128, 1], F32, tag="negm")
                        nc.vector.tensor_scalar_mul(
                            out=negm, in0=out_sb[:, qc : qc + 1], scalar1=-1.0
                        )
                        # z1 partial = sum exp(s1 - chunk max)
                        escr = scrp.tile([128, 512], F32, tag="escr")
                        nc.scalar.activation(
                            out=escr, in_=ps1, func=ACT.Exp, bias=negm, scale=1.0,
                            accum_out=out_sb[:, 8 + qc : 9 + qc],
                        )

                # ------------ Phase B: logit path (256-wide q chunks) ---------
                # Per chunk: one [128, 2, 8192] natural load (256 q rows), PE
                # transposes into a [128, 32, 256] Tbuf in two half-passes of
                # 32 c-chunks each, fp32r matmuls (N=256) accumulate into ps2.
                CB = C // 128  # 64 c-chunks
                with (
                    tc.tile_pool(name="natB", bufs=3) as natB_pool,
                    tc.tile_pool(name="tbuf", bufs=1) as tbuf_pool,
                    tc.tile_pool(name="psum_b", bufs=2, space="PSUM") as psum_b,
                ):
                    for qb in range(B_CHUNKS):
                        ps2 = psum_b.tile([128, 256], F32, tag="ps2")
                        for half in range(2):
                            nat = natB_pool.tile([128, 2, C // 2], B_DT)
                            nc.sync.dma_start(
                                out=nat,
                                in_=logit[
                                    qb * 256 : (qb + 1) * 256,
                                    half * (C // 2) : (half + 1) * (C // 2),
                                ].rearrange("(s p) c -> p s c", p=128),
                            )
                            tb = tbuf_pool.tile([128, CB // 2, 256], B_DT, tag="tb")
                            for g in range(CB // 4):  # pt bank: 2 cb x 2 qsub
                                pt = psum_t.tile([128, 512], F32, tag="pt")
                                for k in range(4):
                                    cbl = g * 2 + k // 2
                                    qs = k % 2
                                    nc.tensor.matmul(
                                        pt[:, k * 128 : (k + 1) * 128],
                                        nat[:, qs, cbl * 128 : (cbl + 1) * 128].bitcast(F32),
                                        ident,
                                        is_transpose=True,
                                        start=(k == 0),
                                        stop=(k == 3),
                                    )
                                dst = tb[:, g * 2 : g * 2 + 2, :]
                                src = pt.rearrange("p (c q) -> p c q", c=2)
                                if g % 2 == 0:
                                    nc.vector.tensor_copy(out=dst, in_=src)
                                else:
                                    nc.scalar.copy(out=dst, in_=src)
                            for gc in range(CB // 2):
                                cb = half * (CB // 2) + gc
                                nc.tensor.matmul(
                                    ps2, nlT_sb[:, cb, :], tb[:, gc, :],
                                    start=(cb == 0), stop=(cb == CB - 1),
                                )
                        qslice = slice(qb * 256, (qb + 1) * 256)
                        # a2 partial
                        scr = scrp.tile([128, 512], F32, tag="scr")
                        nc.vector.scalar_tensor_tensor(
                            out=scr[:, :256], in0=ps2, scalar=1.0,
                            in1=mw_sb[:, qslice], op0=OP.mult, op1=OP.mult,
                            accum_out=out_sb[:, 96 + qb : 97 + qb],
                        )
                        # m2 partial
                        nc.vector.tensor_reduce(
                            out=out_sb[:, 32 + qb : 33 + qb], in_=ps2,
                            axis=AX.X, op=OP.max,
                        )
                        negm = small.tile([128, 1], F32, tag="negm")
                        nc.vector.tensor_scalar_mul(
                            out=negm, in0=out_sb[:, 32 + qb : 33 + qb], scalar1=-1.0
                        )
                        # z2 partial
                        escr = scrp.tile([128, 512], F32, tag="escr")
                        nc.scalar.activation(
                            out=escr[:, :256], in_=ps2, func=ACT.Exp, bias=negm,
                            scale=1.0, accum_out=out_sb[:, 64 + qb : 65 + qb],
                        )

            nc.sync.dma_start(out=stats[:, :], in_=out_sb)

    _split_excess_waits(nc)
    return nc


_PROGRAM: bass.Bass | None = None
LAST_RESULTS = None  # BassKernelResults of the most recent run (for profiling)


def _get_program() -> bass.Bass:
    global _PROGRAM
    if _PROGRAM is None:
        _PROGRAM = _build_program()
    return _PROGRAM


def host_prep(old_embeds, old_logits, new_embeds, new_logits, labels,
              feat_queue, logit_queue, queue_labels, header):
    """Scatter + normalize + mask on host; returns per-core in_maps and M."""
    old_embeds = np.asarray(old_embeds, dtype=np.float32)
    old_logits = np.asarray(old_logits, dtype=np.float32)
    new_embeds = np.asarray(new_embeds, dtype=np.float32)
    new_logits = np.asarray(new_logits, dtype=np.float32)
    feat_queue = np.array(feat_queue, dtype=np.float32)   # copies (scattered below)
    logit_queue = np.array(logit_queue, dtype=np.float32)
    labels_np = np.asarray(labels).astype(np.int64)
    queue_labels_np = np.asarray(queue_labels).astype(np.int64)
    hdr = int(np.asarray(header))

    n = old_embeds.shape[0]
    q = feat_queue.shape[0]
    assert (n, q) == (N, Q)

    # circular queue scatter
    idx = (hdr + np.arange(n)) % q
    feat_queue[idx] = old_embeds
    logit_queue[idx] = old_logits
    queue_labels_np[idx] = labels_np

    # normalize new_embeds (f64 intermediate, f32 result)
    ne64 = new_embeds.astype(np.float64)
    norm = np.sqrt((ne64 * ne64).sum(axis=1, keepdims=True))
    new_e = (ne64 / np.maximum(norm, EPS)).astype(np.float32)

    # label mask (host): maskh = 0.5 * mask
    mask = (queue_labels_np[None, :] == labels_np[:, None])
    M = mask.sum(axis=1).astype(np.float64)               # [N], >= 1 by construction
    maskh = 0.5 * mask.astype(np.float32)

    neT = np.ascontiguousarray(new_e.T)                   # [D, N]
    oeT = np.ascontiguousarray(old_embeds.T)              # [D, N]
    nlT = np.ascontiguousarray(new_logits.T)              # [C, N]

    in_maps = []
    for d in range(N_CORES):
        sl = slice(d * QS, (d + 1) * QS)
        in_maps.append({
            "feat": np.ascontiguousarray(feat_queue[sl]),
            "logit": np.ascontiguousarray(logit_queue[sl]),
            "maskh": np.ascontiguousarray(maskh[:, sl]),
            "neT": neT,
            "oeT": oeT,
            "nlT": nlT,
            "identB": np.eye(128, dtype=np.float32),
        })
    return in_maps, M


def combine_stats(parts: np.ndarray, M: np.ndarray):
    """parts: [n_cores, 128, 128] f32 stats tiles -> (l1, l2) f32 scalars."""
    parts = parts.astype(np.float64)
    m1p = parts[:, :, 0:8]
    z1p = parts[:, :, 8:16]
    a1p = parts[:, :, 16:24]
    wp = parts[:, :, 24:32]
    m2p = parts[:, :, 32 : 32 + B_CHUNKS]
    z2p = parts[:, :, 64 : 64 + B_CHUNKS]
    a2p = parts[:, :, 96 : 96 + B_CHUNKS]

    W = wp.sum(axis=(0, 2))                               # [N]
    A1 = a1p.sum(axis=(0, 2))
    A2 = a2p.sum(axis=(0, 2))
    m1 = m1p.max(axis=(0, 2))
    m2 = m2p.max(axis=(0, 2))
    Z1 = (z1p * np.exp(m1p - m1[None, :, None])).sum(axis=(0, 2))
    Z2 = (z2p * np.exp(m2p - m2[None, :, None])).sum(axis=(0, 2))

    # sum_j maskw * log_prob = A_raw - (m + log Z) * W ; divide by count, mean, negate
    l1 = -np.mean((A1 - (m1 + np.log(Z1)) * W) / M)
    l2 = -np.mean((A2 - (m2 + np.log(Z2)) * W) / M)
    return (np.float32(l1), np.float32(l2))


def kernel(old_embeds, old_logits, new_embeds, new_logits, labels,
           feat_queue, logit_queue, queue_labels, header):
    global LAST_RESULTS
    in_maps, M = host_prep(
        old_embeds, old_logits, new_embeds, new_logits, labels,
        feat_queue, logit_queue, queue_labels, header,
    )
    nc = _get_program()
    LAST_RESULTS = run_bass_kernel_spmd(nc, in_maps, list(range(N_CORES)))
    parts = np.stack([LAST_RESULTS.results[d]["stats"] for d in range(N_CORES)])
    return combine_stats(parts, M)


# revision 32
# speedup vs baseline: 2.8485x; 1.6832x over previous
"""Trainium2 Bass kernel for nn_CompatibleLearningLoss (MoCo-style queue contrastive loss).

Strategy: shard the queue dimension (Q=32768) across 8 NeuronCores (4096 rows each).
Each core computes its slice of the three matmuls
    old_embeds  @ feat_queue_shard.T   -> weight
    new_e       @ feat_queue_shard.T   -> scores1
    new_logits  @ logit_queue_shard.T  -> scores2
and reduces per-row partial softmax statistics (chunk max, exp-sum vs chunk max,
masked-weighted raw sums) into a [128, 128] stats tile.  The host combines the
partials in float64 and produces the two scalar losses.

The circular queue scatter, the new_embeds normalization, the label mask and the
small-matrix transposes are done host-side (they are O(MB) vs the 1GB queue reads).
"""

from contextlib import nullcontext

import numpy as np

import concourse.bass as bass
import concourse.tile as tile
from concourse import mybir
from concourse.bass_utils import run_bass_kernel_spmd
from concourse.masks import make_identity
from concourse.vector_clock import ScopedClock

N = 128      # batch
D = 512      # embed dim
C = 8192     # logit dim
Q = 32768    # queue length
N_CORES = 8
QS = Q // N_CORES          # 4096 queue rows per core
EPS = 1e-12

F32 = mybir.dt.float32

# stats tile column layout (per core, [128, 128] f32)
# cols 0:8      m1 parts   (feat-path chunk maxes, 8 chunks of 512)
# cols 8:16     z1 parts   (feat-path sum exp(s - chunk max))
# cols 16:24    a1 parts   (feat-path sum maskw * s_raw)
# cols 24:32    W  parts   (sum maskw)
# cols 32:32+B  m2 parts   (logit-path chunk maxes, B_CHUNKS chunks)
# cols 64:64+B  z2 parts
# cols 96:96+B  a2 parts
A_CHUNKS = 8    # feat path: 8 chunks of 512 q
B_CHUNKS = 16   # logit path: 16 chunks of 256 q

# precision/perf knobs (validated against the fp32 reference on hardware)
BF16 = mybir.dt.bfloat16
F32R = mybir.dt.float32r
# Logit-path precision knob.
#   BF16: host casts logit_queue/new_logits to bf16 -> halves the dominant DMA
#         (128->64 MB/core) and runs transposes+matmuls at 1 cyc/row.
#   F32R: fp32 storage, reduced-precision multiply (1 cyc/row matmul at N>=256,
#         fp32 2cyc transposes).  NOTE: fp32r *transposes* crash the exec unit
#         (NRT_EXEC_UNIT_UNRECOVERABLE) - keep transposes fp32 in that mode.
# The BIR verifier requires dtype consistency along every producer->consumer
# edge, so the full chain (DRAM logit, nat, tbuf, nlT) shares B_DT.
LOGIT_BF16 = True
B_DT = BF16 if LOGIT_BF16 else F32R


def _split_excess_waits(nc: bass.Bass, limit: int = 1) -> None:
    """This walrus build rejects instructions carrying more than one sync wait
    ("Too many sync wait commands").  Tile's sem-assignment freely attaches
    several.  Move excess waits onto same-engine nops inserted right before
    the offending instruction (queue order makes that equivalent)."""
    for f in nc.m.functions:
        for bb in f.blocks:
            insts = bb.instructions
            insertions = []
            for idx, inst in enumerate(insts):
                si = inst.sync_info
                if si is None:
                    continue
                cap = 2 if isinstance(inst, mybir.InstEventSemaphore) else limit
                waits = list(si.on_wait)
                if len(waits) <= cap:
                    continue
                keep = waits[:cap]
                excess = waits[cap:]
                si.on_wait = keep
                nops = []
                for w in excess:
                    nop = mybir.InstNoOp(
                        name=nc.get_next_instruction_name(), ins=[], outs=[]
                    )
                    nop.engine = inst.engine
                    nop.sync_info = mybir.SyncInfo(on_wait=[w], on_update=[])
                    nc.register_instruction(nop, overwrite=True)
                    nops.append(nop)
                insertions.append((idx, nops))
            for idx, nops in reversed(insertions):
                for nop in reversed(nops):
                    bb.instructions.insert(idx, nop)


class PatchedTileContext(tile.TileContext):
    """Work around the 1-sync-wait-per-instruction cap in this walrus build:
    the stock TileContext tail drain carries one wait per outstanding proc,
    which codegen rejects ("Too many sync wait commands").  Split the waits
    across single-wait SP nops instead."""

    def _drain_and_barrier(self, tick_clock, wait_clock):
        drain_inst = self.nc.sync.drain()
        wait_clock.add_sem_waits(
            drain_inst.ins, ScopedClock({None: tick_clock.global_clock})
        )
        si = drain_inst.ins.sync_info
        if si is not None and len(si.on_wait) > 1:
            waits = list(si.on_wait)
            si.on_wait = [waits[0]]
            for w in waits[1:]:
                nop = self.nc.sync.nop(nofuse=True, hint="drain_wait_split")
                nop.ins.sync_info = mybir.SyncInfo(on_wait=[w], on_update=[])
        self.nc.all_engine_barrier()
        assert self.sems is not None
        popped = self.nc._tile_sem_poison_stack.pop()
        assert popped is self._sem_poison
        self.nc.clear_and_free_semaphores(list(self.sems.allocated().values()))
        self.nc.all_engine_barrier()


def _build_program(repeat: int = 1) -> bass.Bass:
    nc = bass.Bass()

    feat = nc.dram_tensor("feat", [QS, D], F32, kind="ExternalInput")
    logit = nc.dram_tensor("logit", [QS, C], B_DT, kind="ExternalInput")
    maskh = nc.dram_tensor("maskh", [N, QS], F32, kind="ExternalInput")
    neT = nc.dram_tensor("neT", [D, N], F32, kind="ExternalInput")
    oeT = nc.dram_tensor("oeT", [D, N], F32, kind="ExternalInput")
    nlT = nc.dram_tensor("nlT", [C, N], B_DT, kind="ExternalInput")
    identB = nc.dram_tensor("identB", [128, 128], B_DT, kind="ExternalInput")
    stats = nc.dram_tensor("stats", [N, 128], F32, kind="ExternalOutput")

    AX = mybir.AxisListType
    OP = mybir.AluOpType
    ACT = mybir.ActivationFunctionType

    with PatchedTileContext(nc) as tc:
        with (
            tc.tile_pool(name="const", bufs=1) as const,
            tc.tile_pool(name="small", bufs=4) as small,
            tc.tile_pool(name="scr", bufs=2) as scrp,
            tc.tile_pool(name="tsl", bufs=3) as tslp,
            tc.tile_pool(name="psum_t", bufs=3, space="PSUM") as psum_t,
        ):
            ident = const.tile([128, 128], F32)
            make_identity(nc, ident)

            # replicated small operands, pre-transposed host-side
            neT_sb = const.tile([128, D // 128, N], F32)
            nc.sync.dma_start(out=neT_sb, in_=neT.rearrange("(c p) n -> p c n", p=128))
            oeT_sb = const.tile([128, D // 128, N], F32)
            nc.sync.dma_start(out=oeT_sb, in_=oeT.rearrange("(c p) n -> p c n", p=128))
            nlT_sb = const.tile([128, C // 128, N], B_DT)
            nc.sync.dma_start(out=nlT_sb, in_=nlT.rearrange("(c p) n -> p c n", p=128))
            identB_sb = const.tile([128, 128], B_DT)
            nc.sync.dma_start(out=identB_sb, in_=identB[:, :])

            mw_sb = const.tile([N, QS], F32)
            out_sb = const.tile([N, 128], F32)

            loop_cm = tc.For_i(0, repeat, 1) if repeat > 1 else nullcontext()
            with loop_cm:
                # 0.5*mask, becomes maskw = 0.5*(old_outputs+1)*mask in place
                nc.sync.dma_start(out=mw_sb, in_=maskh[:, :])

                # ------------ Phase A: feat path (512-wide q chunks) ----------
                with (
                    tc.tile_pool(name="natA", bufs=2) as natA_pool,
                    tc.tile_pool(name="psum_a", bufs=2, space="PSUM") as psum_a,
                ):
                    for qc in range(A_CHUNKS):
                        nat = natA_pool.tile([128, 4, D], F32)
                        nc.sync.dma_start(
                            out=nat,
                            in_=feat[qc * 512 : (qc + 1) * 512, :].rearrange(
                                "(s p) d -> p s d", p=128
                            ),
                        )
                        ps1 = psum_a.tile([128, 512], F32, tag="ps1")
                        psw = psum_a.tile([128, 512], F32, tag="psw")
                        for dc in range(4):
                            pt = psum_t.tile([128, 512], F32, tag="pt")
                            for qs in range(4):
                                nc.tensor.matmul(
                                    pt[:, qs * 128 : (qs + 1) * 128],
                                    nat[:, qs, dc * 128 : (dc + 1) * 128],
                                    ident,
                                    is_transpose=True,
                                    start=(qs == 0),
                                    stop=(qs == 3),
                                )
                            tsl = tslp.tile([128, 512], F32, tag="tsl")
                            if dc % 2 == 0:
                                nc.vector.tensor_copy(out=tsl, in_=pt)
                            else:
                                nc.scalar.copy(out=tsl, in_=pt)
                            nc.tensor.matmul(
                                ps1, neT_sb[:, dc, :], tsl,
                                start=(dc == 0), stop=(dc == 3),
                            )
                            nc.tensor.matmul(
                                psw, oeT_sb[:, dc, :], tsl,
                                start=(dc == 0), stop=(dc == 3),
                            )
                        qslice = slice(qc * 512, (qc + 1) * 512)
                        # maskw = (w + 1) * 0.5mask, in place over the 0.5mask
                        nc.vector.scalar_tensor_tensor(
                            out=mw_sb[:, qslice], in0=psw, scalar=1.0,
                            in1=mw_sb[:, qslice], op0=OP.add, op1=OP.mult,
                        )
                        # W partial
                        nc.vector.tensor_reduce(
                            out=out_sb[:, 24 + qc : 25 + qc], in_=mw_sb[:, qslice],
                            axis=AX.X, op=OP.add,
                        )
                        # a1 partial = sum maskw * s1_raw
                        scr = scrp.tile([128, 512], F32, tag="scr")
                        nc.vector.scalar_tensor_tensor(
                            out=scr, in0=ps1, scalar=1.0, in1=mw_sb[:, qslice],
                            op0=OP.mult, op1=OP.mult,
                            accum_out=out_sb[:, 16 + qc : 17 + qc],
                        )
                        # m1 partial (chunk max)
                        nc.vector.tensor_reduce(
                            out=out_sb[:, qc : qc + 1], in_=ps1, axis=AX.X, op=OP.max,
                        )
                        negm = small.tile([128, 1], F32, tag="negm")
                        nc.vector.tensor_scalar_mul(
                            out=negm, in0=out_sb[:, qc : qc + 1], scalar1=-1.0
                        )
                        # z1 partial = sum exp(s1 - chunk max)
                        escr = scrp.tile([128, 512], F32, tag="escr")
                        nc.scalar.activation(
                            out=escr, in_=ps1, func=ACT.Exp, bias=negm, scale=1.0,
                            accum_out=out_sb[:, 8 + qc : 9 + qc],
                        )

                # ------------ Phase B: logit path (256-wide q chunks) ---------
                # Per chunk: one [128, 2, 8192] natural load (256 q rows), PE
                # transposes into a [128, 32, 256] Tbuf in two half-passes of
                # 32 c-chunks each, fp32r matmuls (N=256) accumulate into ps2.
                CB = C // 128  # 64 c-chunks
                with (
                    tc.tile_pool(name="natB", bufs=3) as natB_pool,
                    tc.tile_pool(name="tbuf", bufs=1) as tbuf_pool,
                    tc.tile_pool(name="psum_b", bufs=2, space="PSUM") as psum_b,
                ):
                    for qb in range(B_CHUNKS):
                        ps2 = psum_b.tile([128, 256], F32, tag="ps2")
                        for half in range(2):
                            nat = natB_pool.tile([128, 2, C // 2], B_DT)
                            nc.sync.dma_start(
                                out=nat,
                                in_=logit[
                                    qb * 256 : (qb + 1) * 256,
                                    half * (C // 2) : (half + 1) * (C // 2),
                                ].rearrange("(s p) c -> p s c", p=128),
                            )
                            tb = tbuf_pool.tile([128, CB // 2, 256], B_DT, tag="tb")
                            for g in range(CB // 4):  # pt bank: 2 cb x 2 qsub
                                pt = psum_t.tile(
                                    [128, 512], BF16 if LOGIT_BF16 else F32, tag="pt"
                                )
                                for k in range(4):
                                    cbl = g * 2 + k // 2
                                    qs = k % 2
                                    tin = nat[:, qs, cbl * 128 : (cbl + 1) * 128]
                                    if LOGIT_BF16:
                                        tid = identB_sb
                                    else:
                                        tin = tin.bitcast(F32)
                                        tid = ident
                                    nc.tensor.matmul(
                                        pt[:, k * 128 : (k + 1) * 128],
                                        tin,
                                        tid,
                                        is_transpose=True,
                                        start=(k == 0),
                                        stop=(k == 3),
                                    )
                                dst = tb[:, g * 2 : g * 2 + 2, :]
                                src = pt.rearrange("p (c q) -> p c q", c=2)
                                if g % 2 == 0:
                                    nc.vector.tensor_copy(out=dst, in_=src)
                                else:
                                    nc.scalar.copy(out=dst, in_=src)
                            for gc in range(CB // 2):
                                cb = half * (CB // 2) + gc
                                nc.tensor.matmul(
                                    ps2, nlT_sb[:, cb, :], tb[:, gc, :],
                                    start=(cb == 0), stop=(cb == CB - 1),
                                )
                        qslice = slice(qb * 256, (qb + 1) * 256)
                        # a2 partial
                        scr = scrp.tile([128, 512], F32, tag="scr")
                        nc.vector.scalar_tensor_tensor(
                            out=scr[:, :256], in0=ps2, scalar=1.0,
                            in1=mw_sb[:, qslice], op0=OP.mult, op1=OP.mult,
                            accum_out=out_sb[:, 96 + qb : 97 + qb],
                        )
                        # m2 partial
                        nc.vector.tensor_reduce(
                            out=out_sb[:, 32 + qb : 33 + qb], in_=ps2,
                            axis=AX.X, op=OP.max,
                        )
                        negm = small.tile([128, 1], F32, tag="negm")
                        nc.vector.tensor_scalar_mul(
                            out=negm, in0=out_sb[:, 32 + qb : 33 + qb], scalar1=-1.0
                        )
                        # z2 partial
                        escr = scrp.tile([128, 512], F32, tag="escr")
                        nc.scalar.activation(
                            out=escr[:, :256], in_=ps2, func=ACT.Exp, bias=negm,
                            scale=1.0, accum_out=out_sb[:, 64 + qb : 65 + qb],
                        )

            nc.sync.dma_start(out=stats[:, :], in_=out_sb)

    _split_excess_waits(nc)
    return nc


_PROGRAM: bass.Bass | None = None
LAST_RESULTS = None  # BassKernelResults of the most recent run (for profiling)


def _get_program() -> bass.Bass:
    global _PROGRAM
    if _PROGRAM is None:
        _PROGRAM = _build_program()
    return _PROGRAM


def host_prep(old_embeds, old_logits, new_embeds, new_logits, labels,
              feat_queue, logit_queue, queue_labels, header):
    """Scatter + normalize + mask on host; returns per-core in_maps and M."""
    old_embeds = np.asarray(old_embeds, dtype=np.float32)
    old_logits = np.asarray(old_logits, dtype=np.float32)
    new_embeds = np.asarray(new_embeds, dtype=np.float32)
    new_logits = np.asarray(new_logits, dtype=np.float32)
    feat_queue = np.array(feat_queue, dtype=np.float32)   # copies (scattered below)
    logit_queue = np.array(logit_queue, dtype=np.float32)
    labels_np = np.asarray(labels).astype(np.int64)
    queue_labels_np = np.asarray(queue_labels).astype(np.int64)
    hdr = int(np.asarray(header))

    n = old_embeds.shape[0]
    q = feat_queue.shape[0]
    assert (n, q) == (N, Q)

    # circular queue scatter
    idx = (hdr + np.arange(n)) % q
    feat_queue[idx] = old_embeds
    logit_queue[idx] = old_logits
    queue_labels_np[idx] = labels_np

    # normalize new_embeds (f64 intermediate, f32 result)
    ne64 = new_embeds.astype(np.float64)
    norm = np.sqrt((ne64 * ne64).sum(axis=1, keepdims=True))
    new_e = (ne64 / np.maximum(norm, EPS)).astype(np.float32)

    # label mask (host): maskh = 0.5 * mask
    mask = (queue_labels_np[None, :] == labels_np[:, None])
    M = mask.sum(axis=1).astype(np.float64)               # [N], >= 1 by construction
    maskh = 0.5 * mask.astype(np.float32)

    neT = np.ascontiguousarray(new_e.T)                   # [D, N]
    oeT = np.ascontiguousarray(old_embeds.T)              # [D, N]
    nlT = np.ascontiguousarray(new_logits.T)              # [C, N]

    b_np = mybir.dt.np(B_DT)
    nlT_b = np.ascontiguousarray(nlT.astype(b_np))
    ident_b = np.eye(128, dtype=b_np)

    in_maps = []
    for d in range(N_CORES):
        sl = slice(d * QS, (d + 1) * QS)
        in_maps.append({
            "feat": np.ascontiguousarray(feat_queue[sl]),
            "logit": np.ascontiguousarray(logit_queue[sl]).astype(b_np),
            "maskh": np.ascontiguousarray(maskh[:, sl]),
            "neT": neT,
            "oeT": oeT,
            "nlT": nlT_b,
            "identB": ident_b,
        })
    return in_maps, M


def combine_stats(parts: np.ndarray, M: np.ndarray):
    """parts: [n_cores, 128, 128] f32 stats tiles -> (l1, l2) f32 scalars."""
    parts = parts.astype(np.float64)
    m1p = parts[:, :, 0:8]
    z1p = parts[:, :, 8:16]
    a1p = parts[:, :, 16:24]
    wp = parts[:, :, 24:32]
    m2p = parts[:, :, 32 : 32 + B_CHUNKS]
    z2p = parts[:, :, 64 : 64 + B_CHUNKS]
    a2p = parts[:, :, 96 : 96 + B_CHUNKS]

    W = wp.sum(axis=(0, 2))                               # [N]
    A1 = a1p.sum(axis=(0, 2))
    A2 = a2p.sum(axis=(0, 2))
    m1 = m1p.max(axis=(0, 2))
    m2 = m2p.max(axis=(0, 2))
    Z1 = (z1p * np.exp(m1p - m1[None, :, None])).sum(axis=(0, 2))
    Z2 = (z2p * np.exp(m2p - m2[None, :, None])).sum(axis=(0, 2))

    # sum_j maskw * log_prob = A_raw - (m + log Z) * W ; divide by count, mean, negate
    l1 = -np.mean((A1 - (m1 + np.log(Z1)) * W) / M)
    l2 = -np.mean((A2 - (m2 + np.log(Z2)) * W) / M)
    return (np.float32(l1), np.float32(l2))


def kernel(old_embeds, old_logits, new_embeds, new_logits, labels,
           feat_queue, logit_queue, queue_labels, header):
    global LAST_RESULTS
    in_maps, M = host_prep(
        old_embeds, old_logits, new_embeds, new_logits, labels,
        feat_queue, logit_queue, queue_labels, header,
    )
    nc = _get_program()
    LAST_RESULTS = run_bass_kernel_spmd(nc, in_maps, list(range(N_CORES)))
    parts = np.stack([LAST_RESULTS.results[d]["stats"] for d in range(N_CORES)])
    return combine_stats(parts, M)


# revision 34
# speedup vs baseline: 4.0212x; 1.4117x over previous
"""Trainium2 Bass kernel for nn_CompatibleLearningLoss (MoCo-style queue contrastive loss).

Strategy: shard the queue dimension (Q=32768) across 8 NeuronCores (4096 rows
each).  Each core computes its slice of the three matmuls
    old_embeds  @ feat_queue_shard.T   -> weight
    new_e       @ feat_queue_shard.T   -> scores1
    new_logits  @ logit_queue_shard.T  -> scores2
and reduces per-row partial softmax statistics (chunk max, exp-sum vs chunk
max, masked-weighted raw sums) into a [128, 128] stats tile.  The host combines
the partials in float64 and produces the two scalar losses.

Marshaling (all host-side): circular queue scatter, new_embeds normalization,
label mask, and - crucially - pre-transposition of both queue matrices so the
contraction dim lands on SBUF partitions.  The device then runs pure streaming
matmuls with zero on-chip transposes.  The logit queue is cast to bf16 (halves
the dominant DMA stream; measured loss error ~1e-4 relative); the feat path
uses fp32r multiplies (near-fp32, ~5e-6).
"""

from contextlib import nullcontext

import numpy as np

import concourse.bass as bass
import concourse.tile as tile
from concourse import mybir
from concourse.bass_utils import run_bass_kernel_spmd
from concourse.vector_clock import ScopedClock

N = 128      # batch
D = 512      # embed dim
C = 8192     # logit dim
Q = 32768    # queue length
N_CORES = 8
QS = Q // N_CORES          # 4096 queue rows per core
EPS = 1e-12

F32 = mybir.dt.float32
BF16 = mybir.dt.bfloat16
F32R = mybir.dt.float32r

# stats tile column layout (per core, [128, 128] f32)
# cols 0:8      m1 parts   (feat-path chunk maxes, 8 chunks of 512)
# cols 8:16     z1 parts   (feat-path sum exp(s - chunk max))
# cols 16:24    a1 parts   (feat-path sum maskw * s_raw)
# cols 24:32    W  parts   (sum maskw)
# cols 32:32+B  m2 parts   (logit-path chunk maxes, B_CHUNKS chunks of 512)
# cols 64:64+B  z2 parts
# cols 96:96+B  a2 parts
A_CHUNKS = 8
B_CHUNKS = 8


def _split_excess_waits(nc: bass.Bass, limit: int = 1) -> None:
    """This walrus build rejects instructions carrying more than one sync wait
    ("Too many sync wait commands").  Tile's sem-assignment freely attaches
    several.  Move excess waits onto same-engine nops inserted right before
    the offending instruction (queue order makes that equivalent)."""
    for f in nc.m.functions:
        for bb in f.blocks:
            insts = bb.instructions
            insertions = []
            for idx, inst in enumerate(insts):
                si = inst.sync_info
                if si is None:
                    continue
                cap = 2 if isinstance(inst, mybir.InstEventSemaphore) else limit
                waits = list(si.on_wait)
                if len(waits) <= cap:
                    continue
                keep = waits[:cap]
                excess = waits[cap:]
                si.on_wait = keep
                nops = []
                for w in excess:
                    nop = mybir.InstNoOp(
                        name=nc.get_next_instruction_name(), ins=[], outs=[]
                    )
                    nop.engine = inst.engine
                    nop.sync_info = mybir.SyncInfo(on_wait=[w], on_update=[])
                    nc.register_instruction(nop, overwrite=True)
                    nops.append(nop)
                insertions.append((idx, nops))
            for idx, nops in reversed(insertions):
                for nop in reversed(nops):
                    bb.instructions.insert(idx, nop)


class PatchedTileContext(tile.TileContext):
    """Work around the 1-sync-wait-per-instruction cap in this walrus build:
    the stock TileContext tail drain carries one wait per outstanding proc,
    which codegen rejects ("Too many sync wait commands").  Split the waits
    across single-wait SP nops instead."""

    def _drain_and_barrier(self, tick_clock, wait_clock):
        drain_inst = self.nc.sync.drain()
        wait_clock.add_sem_waits(
            drain_inst.ins, ScopedClock({None: tick_clock.global_clock})
        )
        si = drain_inst.ins.sync_info
        if si is not None and len(si.on_wait) > 1:
            waits = list(si.on_wait)
            si.on_wait = [waits[0]]
            for w in waits[1:]:
                nop = self.nc.sync.nop(nofuse=True, hint="drain_wait_split")
                nop.ins.sync_info = mybir.SyncInfo(on_wait=[w], on_update=[])
        self.nc.all_engine_barrier()
        assert self.sems is not None
        popped = self.nc._tile_sem_poison_stack.pop()
        assert popped is self._sem_poison
        self.nc.clear_and_free_semaphores(list(self.sems.allocated().values()))
        self.nc.all_engine_barrier()


def _build_program(repeat: int = 1) -> bass.Bass:
    nc = bass.Bass()

    featT = nc.dram_tensor("featT", [D, QS], F32R, kind="ExternalInput")
    logitT = nc.dram_tensor("logitT", [C, QS], BF16, kind="ExternalInput")
    maskh = nc.dram_tensor("maskh", [N, QS], F32, kind="ExternalInput")
    neT = nc.dram_tensor("neT", [D, N], F32R, kind="ExternalInput")
    oeT = nc.dram_tensor("oeT", [D, N], F32R, kind="ExternalInput")
    nlT = nc.dram_tensor("nlT", [C, N], BF16, kind="ExternalInput")
    stats = nc.dram_tensor("stats", [N, 128], F32, kind="ExternalOutput")

    AX = mybir.AxisListType
    OP = mybir.AluOpType
    ACT = mybir.ActivationFunctionType
    CB = C // 128  # 64 contraction chunks (logit path)

    with PatchedTileContext(nc) as tc:
        with (
            tc.tile_pool(name="const", bufs=1) as const,
            tc.tile_pool(name="small", bufs=4) as small,
            tc.tile_pool(name="scr", bufs=2) as scrp,
        ):
            # replicated stationary operands, pre-transposed host-side
            neT_sb = const.tile([128, D // 128, N], F32R)
            nc.sync.dma_start(out=neT_sb, in_=neT.rearrange("(c p) n -> p c n", p=128))
            oeT_sb = const.tile([128, D // 128, N], F32R)
            nc.sync.dma_start(out=oeT_sb, in_=oeT.rearrange("(c p) n -> p c n", p=128))
            nlT_sb = const.tile([128, CB, N], BF16)
            nc.sync.dma_start(out=nlT_sb, in_=nlT.rearrange("(c p) n -> p c n", p=128))

            mw_sb = const.tile([N, QS], F32)
            out_sb = const.tile([N, 128], F32)

            def stats_chunk(ps_scores, qslice, col_m, col_z, col_a):
                w = qslice.stop - qslice.start
                # a partial = sum maskw * s_raw
                scr = scrp.tile([128, 512], F32, tag="scr")
                nc.vector.scalar_tensor_tensor(
                    out=scr[:, :w], in0=ps_scores, scalar=1.0,
                    in1=mw_sb[:, qslice], op0=OP.mult, op1=OP.mult,
                    accum_out=out_sb[:, col_a : col_a + 1],
                )
                # m partial (chunk max)
                nc.vector.tensor_reduce(
                    out=out_sb[:, col_m : col_m + 1], in_=ps_scores,
                    axis=AX.X, op=OP.max,
                )
                negm = small.tile([128, 1], F32, tag="negm")
                nc.vector.tensor_scalar_mul(
                    out=negm, in0=out_sb[:, col_m : col_m + 1], scalar1=-1.0
                )
                # z partial = sum exp(s - chunk max)
                escr = scrp.tile([128, 512], F32, tag="escr")
                nc.scalar.activation(
                    out=escr[:, :w], in_=ps_scores, func=ACT.Exp, bias=negm,
                    scale=1.0, accum_out=out_sb[:, col_z : col_z + 1],
                )

            loop_cm = tc.For_i(0, repeat, 1) if repeat > 1 else nullcontext()
            with loop_cm:
                # 0.5*mask, becomes maskw = 0.5*(old_outputs+1)*mask in place
                nc.sync.dma_start(out=mw_sb, in_=maskh[:, :])

                # ---- Phase A: feat path (two q halves of 2048, fp32r) -------
                with (
                    tc.tile_pool(name="ftp", bufs=3) as ftp,
                    tc.tile_pool(name="psum_a", bufs=1, space="PSUM") as psum_a,
                ):
                    for qh in range(2):
                        ps1 = [psum_a.tile([128, 512], F32, tag=f"s1_{i}", name=f"s1_{i}")
                               for i in range(4)]
                        psw = [psum_a.tile([128, 512], F32, tag=f"w_{i}", name=f"w_{i}")
                               for i in range(4)]
                        for dc in range(4):
                            ft = ftp.tile([128, 2048], F32R, tag="ft")
                            nc.sync.dma_start(
                                out=ft,
                                in_=featT[dc * 128 : (dc + 1) * 128,
                                          qh * 2048 : (qh + 1) * 2048],
                            )
                            for qw in range(4):
                                rhs = ft[:, qw * 512 : (qw + 1) * 512]
                                nc.tensor.matmul(
                                    ps1[qw], neT_sb[:, dc, :], rhs,
                                    start=(dc == 0), stop=(dc == 3),
                                )
                                nc.tensor.matmul(
                                    psw[qw], oeT_sb[:, dc, :], rhs,
                                    start=(dc == 0), stop=(dc == 3),
                                )
                        for qw in range(4):
                            qc = qh * 4 + qw
                            qslice = slice(qc * 512, (qc + 1) * 512)
                            # maskw = (w + 1) * 0.5mask, in place
                            nc.vector.scalar_tensor_tensor(
                                out=mw_sb[:, qslice], in0=psw[qw], scalar=1.0,
                                in1=mw_sb[:, qslice], op0=OP.add, op1=OP.mult,
                            )
                            # W partial
                            nc.vector.tensor_reduce(
                                out=out_sb[:, 24 + qc : 25 + qc],
                                in_=mw_sb[:, qslice], axis=AX.X, op=OP.add,
                            )
                            stats_chunk(ps1[qw], qslice, qc, 8 + qc, 16 + qc)

                # ---- Phase B: logit path (bf16, 8 psum accumulators) --------
                with (
                    tc.tile_pool(name="tcp", bufs=4) as tcp,
                    tc.tile_pool(name="psum_b", bufs=1, space="PSUM") as psum_b,
                ):
                    psB = [psum_b.tile([128, 512], F32, tag=f"b_{i}", name=f"b_{i}")
                           for i in range(8)]
                    for cb in range(CB):
                        tcb = tcp.tile([128, QS], BF16, tag="tcb")
                        nc.sync.dma_start(
                            out=tcb, in_=logitT[cb * 128 : (cb + 1) * 128, :]
                        )
                        for qw in range(8):
                            nc.tensor.matmul(
                                psB[qw], nlT_sb[:, cb, :],
                                tcb[:, qw * 512 : (qw + 1) * 512],
                                start=(cb == 0), stop=(cb == CB - 1),
                            )
                    for qw in range(8):
                        qslice = slice(qw * 512, (qw + 1) * 512)
                        stats_chunk(psB[qw], qslice, 32 + qw, 64 + qw, 96 + qw)

            nc.sync.dma_start(out=stats[:, :], in_=out_sb)

    _split_excess_waits(nc)
    return nc


_PROGRAM: bass.Bass | None = None
LAST_RESULTS = None  # BassKernelResults of the most recent run (for profiling)


def _get_program() -> bass.Bass:
    global _PROGRAM
    if _PROGRAM is None:
        _PROGRAM = _build_program()
    return _PROGRAM


def _transpose_cast_bf16(a: np.ndarray) -> np.ndarray:
    """[R, C] f32 -> [C, R] bf16 (ml_dtypes), contiguous.  Torch does the
    blocked transpose ~5x faster than numpy; fall back to numpy if absent."""
    import ml_dtypes

    try:
        import torch

        t = torch.from_numpy(np.ascontiguousarray(a))
        lt = t.to(torch.bfloat16).t().contiguous()
        return lt.view(torch.int16).numpy().view(ml_dtypes.bfloat16)
    except ImportError:
        return np.ascontiguousarray(a.astype(ml_dtypes.bfloat16).T)


def host_prep(old_embeds, old_logits, new_embeds, new_logits, labels,
              feat_queue, logit_queue, queue_labels, header):
    """Scatter + normalize + mask + pre-transpose on host; returns per-core
    in_maps and the per-row positive counts M."""
    import ml_dtypes

    old_embeds = np.asarray(old_embeds, dtype=np.float32)
    old_logits = np.asarray(old_logits, dtype=np.float32)
    new_embeds = np.asarray(new_embeds, dtype=np.float32)
    new_logits = np.asarray(new_logits, dtype=np.float32)
    feat_queue = np.array(feat_queue, dtype=np.float32)   # copies (scattered below)
    logit_queue = np.array(logit_queue, dtype=np.float32)
    labels_np = np.asarray(labels).astype(np.int64)
    queue_labels_np = np.asarray(queue_labels).astype(np.int64)
    hdr = int(np.asarray(header))

    n = old_embeds.shape[0]
    q = feat_queue.shape[0]
    assert (n, q) == (N, Q)

    # circular queue scatter
    idx = (hdr + np.arange(n)) % q
    feat_queue[idx] = old_embeds
    logit_queue[idx] = old_logits
    queue_labels_np[idx] = labels_np

    # normalize new_embeds (f64 intermediate, f32 result)
    ne64 = new_embeds.astype(np.float64)
    norm = np.sqrt((ne64 * ne64).sum(axis=1, keepdims=True))
    new_e = (ne64 / np.maximum(norm, EPS)).astype(np.float32)

    # label mask (host): maskh = 0.5 * mask
    mask = (queue_labels_np[None, :] == labels_np[:, None])
    M = mask.sum(axis=1).astype(np.float64)               # [N], >= 1 by construction
    maskh = 0.5 * mask.astype(np.float32)

    neT = np.ascontiguousarray(new_e.T)                   # [D, N] f32
    oeT = np.ascontiguousarray(old_embeds.T)              # [D, N] f32
    nlT_b = _transpose_cast_bf16(new_logits)              # [C, N] bf16

    in_maps = []
    for d in range(N_CORES):
        sl = slice(d * QS, (d + 1) * QS)
        in_maps.append({
            "featT": np.ascontiguousarray(feat_queue[sl].T),   # [D, QS] f32
            "logitT": _transpose_cast_bf16(logit_queue[sl]),   # [C, QS] bf16
            "maskh": np.ascontiguousarray(maskh[:, sl]),
            "neT": neT,
            "oeT": oeT,
            "nlT": nlT_b,
        })
    return in_maps, M


def combine_stats(parts: np.ndarray, M: np.ndarray):
    """parts: [n_cores, 128, 128] f32 stats tiles -> (l1, l2) f32 scalars."""
    parts = parts.astype(np.float64)
    m1p = parts[:, :, 0:8]
    z1p = parts[:, :, 8:16]
    a1p = parts[:, :, 16:24]
    wp = parts[:, :, 24:32]
    m2p = parts[:, :, 32 : 32 + B_CHUNKS]
    z2p = parts[:, :, 64 : 64 + B_CHUNKS]
    a2p = parts[:, :, 96 : 96 + B_CHUNKS]

    W = wp.sum(axis=(0, 2))                               # [N]
    A1 = a1p.sum(axis=(0, 2))
    A2 = a2p.sum(axis=(0, 2))
    m1 = m1p.max(axis=(0, 2))
    m2 = m2p.max(axis=(0, 2))
    Z1 = (z1p * np.exp(m1p - m1[None, :, None])).sum(axis=(0, 2))
    Z2 = (z2p * np.exp(m2p - m2[None, :, None])).sum(axis=(0, 2))

    # sum_j maskw * log_prob = A_raw - (m + log Z) * W ; divide by count, mean, negate
    l1 = -np.mean((A1 - (m1 + np.log(Z1)) * W) / M)
    l2 = -np.mean((A2 - (m2 + np.log(Z2)) * W) / M)
    return (np.float32(l1), np.float32(l2))


def kernel(old_embeds, old_logits, new_embeds, new_logits, labels,
           feat_queue, logit_queue, queue_labels, header):
    global LAST_RESULTS
    in_maps, M = host_prep(
        old_embeds, old_logits, new_embeds, new_logits, labels,
        feat_queue, logit_queue, queue_labels, header,
    )
    nc = _get_program()
    LAST_RESULTS = run_bass_kernel_spmd(nc, in_maps, list(range(N_CORES)))
    parts = np.stack([LAST_RESULTS.results[d]["stats"] for d in range(N_CORES)])
    return combine_stats(parts, M)


# revision 51
# speedup vs baseline: 4.5940x; 1.1425x over previous
"""Trainium2 Bass kernel for nn_CompatibleLearningLoss (MoCo-style queue contrastive loss).

Strategy: shard the queue dimension (Q=32768) across 8 NeuronCores (4096 rows
each).  Each core computes its slice of the three matmuls
    old_embeds  @ feat_queue_shard.T   -> weight
    new_e       @ feat_queue_shard.T   -> scores1
    new_logits  @ logit_queue_shard.T  -> scores2
and reduces per-row partial softmax statistics (chunk max, exp-sum vs chunk
max, masked-weighted raw sums) into a [128, 128] stats tile.  The host combines
the partials in float64 and produces the two scalar losses.

Marshaling (all host-side): circular queue scatter, new_embeds normalization,
label mask, and - crucially - pre-transposition of both queue matrices so the
contraction dim lands on SBUF partitions.  The device then runs pure streaming
matmuls with zero on-chip transposes.  The logit queue is cast to bf16 (halves
the dominant DMA stream; measured loss error ~1e-4 relative); the feat path
uses fp32r multiplies (near-fp32, ~5e-6).
"""

from contextlib import nullcontext

import numpy as np

import concourse.bass as bass
import concourse.tile as tile
from concourse import mybir
from concourse.bass_utils import run_bass_kernel_spmd
from concourse.vector_clock import ScopedClock

N = 128      # batch
D = 512      # embed dim
C = 8192     # logit dim
Q = 32768    # queue length
N_CORES = 8
QS = Q // N_CORES          # 4096 queue rows per core
EPS = 1e-12

F32 = mybir.dt.float32
BF16 = mybir.dt.bfloat16
F32R = mybir.dt.float32r

# stats tile column layout (per core, [128, 128] f32)
# cols 0:8      m1 parts   (feat-path chunk maxes, 8 chunks of 512)
# cols 8:16     z1 parts   (feat-path sum exp(s - chunk max))
# cols 16:24    a1 parts   (feat-path sum maskw * s_raw)
# cols 24:32    W  parts   (sum maskw)
# cols 32:32+B  m2 parts   (logit-path chunk maxes, B_CHUNKS chunks of 512)
# cols 64:64+B  z2 parts
# cols 96:96+B  a2 parts
A_CHUNKS = 8
B_CHUNKS = 8


def _split_excess_waits(nc: bass.Bass, limit: int = 1) -> None:
    """This walrus build rejects instructions carrying more than one sync wait
    ("Too many sync wait commands").  Tile's sem-assignment freely attaches
    several.  Move excess waits onto same-engine nops inserted right before
    the offending instruction (queue order makes that equivalent)."""
    for f in nc.m.functions:
        for bb in f.blocks:
            insts = bb.instructions
            insertions = []
            for idx, inst in enumerate(insts):
                si = inst.sync_info
                if si is None:
                    continue
                cap = 2 if isinstance(inst, mybir.InstEventSemaphore) else limit
                waits = list(si.on_wait)
                if len(waits) <= cap:
                    continue
                keep = waits[:cap]
                excess = waits[cap:]
                si.on_wait = keep
                nops = []
                for w in excess:
                    nop = mybir.InstNoOp(
                        name=nc.get_next_instruction_name(), ins=[], outs=[]
                    )
                    nop.engine = inst.engine
                    nop.sync_info = mybir.SyncInfo(on_wait=[w], on_update=[])
                    nc.register_instruction(nop, overwrite=True)
                    nops.append(nop)
                insertions.append((idx, nops))
            for idx, nops in reversed(insertions):
                for nop in reversed(nops):
                    bb.instructions.insert(idx, nop)


class PatchedTileContext(tile.TileContext):
    """Work around the 1-sync-wait-per-instruction cap in this walrus build:
    the stock TileContext tail drain carries one wait per outstanding proc,
    which codegen rejects ("Too many sync wait commands").  Split the waits
    across single-wait SP nops instead."""

    def _drain_and_barrier(self, tick_clock, wait_clock):
        drain_inst = self.nc.sync.drain()
        wait_clock.add_sem_waits(
            drain_inst.ins, ScopedClock({None: tick_clock.global_clock})
        )
        si = drain_inst.ins.sync_info
        if si is not None and len(si.on_wait) > 1:
            waits = list(si.on_wait)
            si.on_wait = [waits[0]]
            for w in waits[1:]:
                nop = self.nc.sync.nop(nofuse=True, hint="drain_wait_split")
                nop.ins.sync_info = mybir.SyncInfo(on_wait=[w], on_update=[])
        self.nc.all_engine_barrier()
        assert self.sems is not None
        popped = self.nc._tile_sem_poison_stack.pop()
        assert popped is self._sem_poison
        self.nc.clear_and_free_semaphores(list(self.sems.allocated().values()))
        self.nc.all_engine_barrier()


def _build_program(repeat: int = 1) -> bass.Bass:
    nc = bass.Bass()

    featT = nc.dram_tensor("featT", [D, QS], F32R, kind="ExternalInput")
    logitT = nc.dram_tensor("logitT", [C, QS], BF16, kind="ExternalInput")
    maskh = nc.dram_tensor("maskh", [N, QS], F32, kind="ExternalInput")
    # stationary operands pre-arranged host-side as their SBUF image
    # [partition, chunk, batch] so each DMA is one contiguous run per partition
    neT = nc.dram_tensor("neT", [128, D // 128, N], F32R, kind="ExternalInput")
    oeT = nc.dram_tensor("oeT", [128, D // 128, N], F32R, kind="ExternalInput")
    nlT = nc.dram_tensor("nlT", [128, C // 128, N], BF16, kind="ExternalInput")
    stats = nc.dram_tensor("stats", [N, 128], F32, kind="ExternalOutput")

    AX = mybir.AxisListType
    OP = mybir.AluOpType
    ACT = mybir.ActivationFunctionType
    CB = C // 128  # 64 contraction chunks (logit path)

    with PatchedTileContext(nc) as tc:
        with (
            tc.tile_pool(name="const", bufs=1) as const,
            tc.tile_pool(name="small", bufs=4) as small,
            tc.tile_pool(name="scr", bufs=2) as scrp,
            tc.tile_pool(name="ftp", bufs=8) as ftp,
        ):
            # replicated stationary operands, pre-transposed host-side
            neT_sb = const.tile([128, D // 128, N], F32R)
            nc.sync.dma_start(out=neT_sb, in_=neT[:, :, :])
            oeT_sb = const.tile([128, D // 128, N], F32R)
            nc.sync.dma_start(out=oeT_sb, in_=oeT[:, :, :])
            nlT_sb = const.tile([128, CB, N], BF16)
            nc.sync.dma_start(out=nlT_sb, in_=nlT[:, :, :])

            mw_sb = const.tile([N, QS], F32)
            out_sb = const.tile([N, 128], F32)
            s1c_sb = const.tile([N, A_CHUNKS, 512], F32)  # scores1 parked in SBUF

            def stats_block(src, col_m, col_z, col_a, nch, mw3):
                """Per-row stats over a [128, nch, 512] block `src`:
                chunk maxes -> cols [col_m, col_m+nch), exp-sums vs chunk max
                -> cols [col_z, ...), one total masked-weighted raw sum ->
                col_a.  mw3 is the matching [128, nch, 512] maskw view."""
                nc.vector.tensor_reduce(
                    out=out_sb[:, col_m : col_m + nch], in_=src,
                    axis=AX.X, op=OP.max,
                )
                negm = small.tile([128, 8], F32, tag="negm")
                nc.vector.tensor_scalar_mul(
                    out=negm[:, :nch], in0=out_sb[:, col_m : col_m + nch],
                    scalar1=-1.0,
                )
                for k in range(nch):
                    escr = scrp.tile([128, 512], F32, tag="escr")
                    nc.scalar.activation(
                        out=escr, in_=src[:, k, :], func=ACT.Exp,
                        bias=negm[:, k : k + 1], scale=1.0,
                        accum_out=out_sb[:, col_z + k : col_z + k + 1],
                    )
                # a partial = sum maskw * s_raw (product scratch: s1c_sb)
                nc.vector.scalar_tensor_tensor(
                    out=s1c_sb[:, :nch, :], in0=src, scalar=1.0, in1=mw3,
                    op0=OP.mult, op1=OP.mult,
                    accum_out=out_sb[:, col_a : col_a + 1],
                )

            loop_cm = tc.For_i(0, repeat, 1) if repeat > 1 else nullcontext()
            with loop_cm:
                # 0.5*mask, becomes maskw = 0.5*(old_outputs+1)*mask in place
                nc.sync.dma_start(out=mw_sb, in_=maskh[:, :])

                # ---- Phase A: feat path (two q halves of 2048, fp32r).
                # PSUM results are parked in SBUF immediately (scores1) or
                # consumed by one DVE op (weight -> maskw) so the banks free
                # fast; the ACT-side stats run later, overlapped with phase B's
                # DMA stream, keeping the two HWDGE rings unobstructed.
                with tc.tile_pool(name="psum_a", bufs=1, space="PSUM") as psum_a:
                    for qh in range(2):
                        ps1 = psum_a.tile([128, 4, 512], F32, tag="ps1")
                        psw = psum_a.tile([128, 4, 512], F32, tag="psw")
                        for dc in range(4):
                            ft = ftp.tile([128, 2048], F32R, tag="ft")
                            dma_eng = nc.sync if dc % 2 == 0 else nc.scalar
                            dma_eng.dma_start(
                                out=ft,
                                in_=featT[dc * 128 : (dc + 1) * 128,
                                          qh * 2048 : (qh + 1) * 2048],
                            )
                            for qw in range(4):
                                rhs = ft[:, qw * 512 : (qw + 1) * 512]
                                nc.tensor.matmul(
                                    ps1[:, qw, :], neT_sb[:, dc, :], rhs,
                                    start=(dc == 0), stop=(dc == 3),
                                )
                                nc.tensor.matmul(
                                    psw[:, qw, :], oeT_sb[:, dc, :], rhs,
                                    start=(dc == 0), stop=(dc == 3),
                                )
                        hs = slice(qh * 2048, (qh + 1) * 2048)
                        # maskw = (w + 1) * 0.5mask, in place
                        nc.vector.scalar_tensor_tensor(
                            out=mw_sb[:, hs].rearrange("p (c q) -> p c q", c=4),
                            in0=psw, scalar=1.0,
                            in1=mw_sb[:, hs].rearrange("p (c q) -> p c q", c=4),
                            op0=OP.add, op1=OP.mult,
                        )
                        # park scores1 in SBUF so the banks free fast
                        nc.vector.tensor_copy(
                            out=s1c_sb[:, qh * 4 : (qh + 1) * 4, :], in_=ps1
                        )

                # ---- Phase B: logit path (bf16, 8 psum accumulators) --------
                with (
                    tc.tile_pool(name="tcp", bufs=8) as tcp,
                    tc.tile_pool(name="psum_b", bufs=1, space="PSUM") as psum_b,
                ):
                    psB = psum_b.tile([128, 8, 512], F32, tag="psB")
                    mw3 = mw_sb.rearrange("p (c q) -> p c q", c=8)

                    def b_step(cb):
                        tcb = tcp.tile([128, QS], BF16, tag="tcb", name="tcb")
                        dma_eng = nc.sync if cb % 2 == 0 else nc.scalar
                        dma_eng.dma_start(
                            out=tcb, in_=logitT[cb * 128 : (cb + 1) * 128, :]
                        )
                        for qw in range(8):
                            nc.tensor.matmul(
                                psB[:, qw, :], nlT_sb[:, cb, :],
                                tcb[:, qw * 512 : (qw + 1) * 512],
                                start=(cb == 0), stop=(cb == CB - 1),
                            )

                    for cb in range(24):
                        b_step(cb)
                    # deferred phase A stats, interleaved mid-stream so the
                    # DVE/ACT work hides under the logit DMA stream
                    nc.vector.tensor_reduce(
                        out=out_sb[:, 24:32], in_=mw3, axis=AX.X, op=OP.add,
                    )
                    stats_block(s1c_sb, 0, 8, 16, A_CHUNKS, mw3)
                    for cb in range(24, CB):
                        b_step(cb)
                    # phase B stats
                    stats_block(psB, 32, 64, 96, 8, mw3)

            nc.sync.dma_start(out=stats[:, :], in_=out_sb)

    _split_excess_waits(nc)
    return nc


_PROGRAM: bass.Bass | None = None
LAST_RESULTS = None  # BassKernelResults of the most recent run (for profiling)


def _get_program() -> bass.Bass:
    global _PROGRAM
    if _PROGRAM is None:
        _PROGRAM = _build_program()
    return _PROGRAM


def _transpose_cast_bf16(a: np.ndarray) -> np.ndarray:
    """[R, C] f32 -> [C, R] bf16 (ml_dtypes), contiguous.  Torch does the
    blocked transpose ~5x faster than numpy; fall back to numpy if absent."""
    import ml_dtypes

    try:
        import torch

        t = torch.from_numpy(np.ascontiguousarray(a))
        lt = t.to(torch.bfloat16).t().contiguous()
        return lt.view(torch.int16).numpy().view(ml_dtypes.bfloat16)
    except ImportError:
        return np.ascontiguousarray(a.astype(ml_dtypes.bfloat16).T)


def host_prep(old_embeds, old_logits, new_embeds, new_logits, labels,
              feat_queue, logit_queue, queue_labels, header):
    """Scatter + normalize + mask + pre-transpose on host; returns per-core
    in_maps and the per-row positive counts M."""
    import ml_dtypes

    old_embeds = np.asarray(old_embeds, dtype=np.float32)
    old_logits = np.asarray(old_logits, dtype=np.float32)
    new_embeds = np.asarray(new_embeds, dtype=np.float32)
    new_logits = np.asarray(new_logits, dtype=np.float32)
    feat_queue = np.array(feat_queue, dtype=np.float32)   # copies (scattered below)
    logit_queue = np.array(logit_queue, dtype=np.float32)
    labels_np = np.asarray(labels).astype(np.int64)
    queue_labels_np = np.asarray(queue_labels).astype(np.int64)
    hdr = int(np.asarray(header))

    n = old_embeds.shape[0]
    q = feat_queue.shape[0]
    assert (n, q) == (N, Q)

    # circular queue scatter
    idx = (hdr + np.arange(n)) % q
    feat_queue[idx] = old_embeds
    logit_queue[idx] = old_logits
    queue_labels_np[idx] = labels_np

    # normalize new_embeds (f64 intermediate, f32 result)
    ne64 = new_embeds.astype(np.float64)
    norm = np.sqrt((ne64 * ne64).sum(axis=1, keepdims=True))
    new_e = (ne64 / np.maximum(norm, EPS)).astype(np.float32)

    # label mask (host): maskh = 0.5 * mask
    mask = (queue_labels_np[None, :] == labels_np[:, None])
    M = mask.sum(axis=1).astype(np.float64)               # [N], >= 1 by construction
    maskh = 0.5 * mask.astype(np.float32)

    def _sbuf_image(aT):
        # [K, N] -> [128, K//128, N] partition-major SBUF image
        k = aT.shape[0]
        return np.ascontiguousarray(
            aT.reshape(k // 128, 128, aT.shape[1]).transpose(1, 0, 2)
        )

    neT = _sbuf_image(np.ascontiguousarray(new_e.T))          # [128, 4, N] f32
    oeT = _sbuf_image(np.ascontiguousarray(old_embeds.T))     # [128, 4, N] f32
    nlT_b = _sbuf_image(_transpose_cast_bf16(new_logits))     # [128, 64, N] bf16

    in_maps = []
    for d in range(N_CORES):
        sl = slice(d * QS, (d + 1) * QS)
        in_maps.append({
            "featT": np.ascontiguousarray(feat_queue[sl].T),   # [D, QS] f32
            "logitT": _transpose_cast_bf16(logit_queue[sl]),   # [C, QS] bf16
            "maskh": np.ascontiguousarray(maskh[:, sl]),
            "neT": neT,
            "oeT": oeT,
            "nlT": nlT_b,
        })
    return in_maps, M


def combine_stats(parts: np.ndarray, M: np.ndarray):
    """parts: [n_cores, 128, 128] f32 stats tiles -> (l1, l2) f32 scalars."""
    parts = parts.astype(np.float64)
    m1p = parts[:, :, 0:8]
    z1p = parts[:, :, 8:16]
    a1p = parts[:, :, 16:17]
    wp = parts[:, :, 24:32]
    m2p = parts[:, :, 32 : 32 + B_CHUNKS]
    z2p = parts[:, :, 64 : 64 + B_CHUNKS]
    a2p = parts[:, :, 96:97]

    W = wp.sum(axis=(0, 2))                               # [N]
    A1 = a1p.sum(axis=(0, 2))
    A2 = a2p.sum(axis=(0, 2))
    m1 = m1p.max(axis=(0, 2))
    m2 = m2p.max(axis=(0, 2))
    Z1 = (z1p * np.exp(m1p - m1[None, :, None])).sum(axis=(0, 2))
    Z2 = (z2p * np.exp(m2p - m2[None, :, None])).sum(axis=(0, 2))

    # sum_j maskw * log_prob = A_raw - (m + log Z) * W ; divide by count, mean, negate
    l1 = -np.mean((A1 - (m1 + np.log(Z1)) * W) / M)
    l2 = -np.mean((A2 - (m2 + np.log(Z2)) * W) / M)
    return (np.float32(l1), np.float32(l2))


def kernel(old_embeds, old_logits, new_embeds, new_logits, labels,
           feat_queue, logit_queue, queue_labels, header):
    global LAST_RESULTS
    in_maps, M = host_prep(
        old_embeds, old_logits, new_embeds, new_logits, labels,
        feat_queue, logit_queue, queue_labels, header,
    )
    nc = _get_program()
    LAST_RESULTS = run_bass_kernel_spmd(nc, in_maps, list(range(N_CORES)))
    parts = np.stack([LAST_RESULTS.results[d]["stats"] for d in range(N_CORES)])
    return combine_stats(parts, M)
